# revision 33
# baseline (speedup 1.0000x reference)
"""Multi-head attention kernel for Trainium2, SPMD across 8 NeuronCores.

Problem: q,k,v [B=2, H=16, S=2048, D=64] f32;
  out = softmax(q @ k^T / sqrt(4)) @ v      (scale quirk: d_k = tensor RANK = 4)

Sharding: 32 (b,h) heads split 4-per-core across 8 cores; the forward pass is
fully data-parallel (no collectives).

v2 changes over the original (HW-calibrated via microbenchmarks; the axon
deployment has no NTFF hook, so per-engine rates were measured with
slope-method microbenches against the TimelineSim cost model — PE fp32r
0.81x model, ACT 0.83x, DVE 0.99x, GPSIMD 2.14x SLOWER, 256B-chunk strided
DMA 3.1x SLOWER):
  - DMA: the kernel-layout gather "(n p) d -> p n d" moves 256B chunks per
    descriptor and measured ~3.1x slower than modeled (~57GB/s/core). All
    input/output DMAs now use a PERMUTED sequence order s = n*1024 + p*8 + r
    ("(n p r) d -> p n r d"), which makes every DMA move 2KB-contiguous
    DRAM chunks per partition. Attention is permutation-invariant along t
    (softmax sums over all keys), and the q-permutation is undone by the
    output DMA's matching rearrange, so results are unchanged.
  - The Schraudolph fast-exp path wrote int32 via DVE then bounced through a
    GPSIMD tensor_copy to get f32r-typed bits; GPSIMD copies measured 2.1x
    slower than modeled (~1.9us per [128,512] tile), making Pool a ~119us
    serial lane. The bounce is gone: DVE writes straight into the fp32r tile
    through a dst bitcast (PE reads TF32-truncated bits; harmless vs the
    fast-exp's ~3% error).
  - Scores for the two packed heads now land in ONE two-bank PSUM tile
    [128, 1024], so each t-iter needs a single exp instruction (ACT fixed
    overhead is ~172+ cycles/instruction; halving instruction count saves
    ~25% ACT busy time).
  - PV matmuls run in bf16 (same 1 cycle/row PE rate as fp32r). This lets
    the fast-exp path write int16 bf16-bit-patterns via a dst bitcast --
    the BIR verifier rejects non-f32r-rounding producers for fp32r matmul
    inputs, which is what forced the GPSIMD bounce originally. bf16 P/V
    adds ~0.2-0.4% elementwise error, small vs the fast-exp's ~3%.
  - Per-head outputs are staged in SBUF across all 4 q-chunks and stored
    as one 512KB DMA per head (2KB chunks) at pair end.

v3/v4 changes (second optimization round):
  - QK matmuls are now FULL-HEIGHT: K^T is stored split-padded per head
    (kta = [ktA ; 0], ktb = [0 ; ktB], pads zeroed once at startup), with
    the packed Q^T as the shared 128-partition moving operand -- the zero
    rows annul the other head. Microbenches showed 64-row half-height
    stationaries cost ~2x per row on HW (320ns vs 173ns per 512-row
    matmul) while alternating full-height stationaries is free, so the
    same matmul cycles buy twice the throughput. (The original design
    assumed the two heads' half-height matmuls run concurrently on
    disjoint PE row groups; on this hardware they do not.)
  - V1 builds moved to the otherwise-idle GPSIMD engine (SBUF-to-SBUF
    only; GPSIMD cannot touch PSUM).
  - Head A's O-accumulator is double-buffered (the epilogue-transpose
    staging shares ps_t's bank via a same-tag tile, freeing one), so the
    next q-chunk's first PV matmul doesn't wait on the PSUM-release copy.

v5 changes (third round): the per-pair work is one global iteration
stream -- PV lags the scores by 2 iterations ACROSS q-chunk boundaries
(no pipeline drain/refill at the 3 interior boundaries), and the epilogue
is split: PSUM-release copies issue right after each chunk's last PV,
while the PE transposes+normalize run 4 iterations later (slot t=5 of the
next chunk) so they never block the next scores matmuls in PE program
order.

Measured (slope method over a 1-rep vs 25-rep NEFF, outlier-robust
anchored medians; see test.py): ~129-142us/invocation across runs (the
shared machine's quiet-state floor itself drifts a few hundred us per
call between runs) vs ~333-386us for the original; rel err (max/max)
~1.1e-2. Also probed: f32r-typed Q/K staging to get 1.5-cycle/row
transposes -- the required casting DMA only exists on the GPSIMD DGE
queue and costs more than the 3.4us of PE it saves. Note t(reps) is measurably
non-linear: a 25-vs-49-rep contrast reads ~167us/rep (larger fully
unrolled NEFFs run slower per rep -- instruction-fetch or DVFS effects),
so slope numbers are only comparable at matched rep counts. TimelineSim per-rep slope is 132us
with PE.ENGINE busy 121us/rep: tensor-engine-bound, within ~10us of the
model. Probed and ruled out: LoadStationary churn (free, even full-height
alternating), fp8/DoubleRow (error budget), QCHUNK=256 for more PSUM
elasticity (per-instruction overhead dominates). Tried and reverted:
deferring per-qc epilogue transposes into the next chunk's PE slack slots
(the deferred transposes stall on the PSUM-release copy in program order
ahead of the next scores matmuls; measured slower).

Per-core algorithm (flash-attention style, scores kept TRANSPOSED so the
probability tiles come out already in the orientation the P@V matmul needs):
  - Build paired Q^T, K^T [128, S] (partitions 0-63 head A's [d, s], 64-127
    head B's) via TensorE transposes whose free axis is (head, d): one
    [128,128] transpose per s-tile lands both heads at once. The transposes
    are emitted as work units interleaved into the PREVIOUS head-pair's main
    loop so they ride in PE slack cycles. The two heads' QK^T matmuls target
    disjoint PE row groups (tile_position from base partition) and run
    CONCURRENTLY, halving the K=64 score-matmul wall time.
  - For each q-chunk (512 q) and t-tile (128 t):
      S^T[t, (A q | B q)] = K^T_tile.T @ Q^T_chunk   (two matmuls into one
                                                      2-bank PSUM tile)
      P^T = exp(0.5 * S^T)          one [128,1024] instruction; most t-tiles
                                    on ScalarE Exp, a subset on VectorE
                                    Schraudolph fast-exp (s*EXPA+EXPB ->
                                    int32 bits reinterpreted as f32, ~3% max
                                    elementwise error) to balance engine load
      O^T[d+2, q] += V1_tile.T @ P^T_head  (V1 = [V | ones | ones]: row 64
                                            accumulates the softmax
                                            denominator for free)
    PV matmuls are software-pipelined one t-tile behind the scores matmuls.
  - Epilogue per q-chunk: transpose O^T back to [q, 66] via TensorE,
    multiply by reciprocal(denominator) on VectorE, DMA out (1KB chunks).

The big matmuls run in float32r (TF32-style fast fp32: 1 cycle/row vs 4 for
plain fp32; microbenched at full rate). fp32r ISA restrictions: even
innermost free counts, 8B-aligned dst offsets, dst start_partition 0 —
hence V1 padded to 66 columns.

No max-subtraction in the softmax: scaled scores are ~N(0, 4) with |s| < ~25
for these inputs, so exp stays in f32 range and softmax's scale invariance
cancels any constant bias.
"""

import numpy as np

B, H, S, D = 2, 16, 2048, 64
N_CORES = 8
HPC = (B * H) // N_CORES  # heads per core = 4
P = 128
RB = 8  # DRAM rows per partition chunk (2KB) in the permuted layout
NB = S // (P * RB)  # n blocks = 2
T_TILES = S // P  # 16
QCHUNK = 512
N_QCHUNKS = S // QCHUNK  # 4
VE = D + 2  # V1 columns: 64 data + 1 ones (denominator) + 1 pad
SCALE = 0.5  # 1/sqrt(d_k) with d_k = k.ndim = 4 (faithful to reference)
# Schraudolph fast-exp constants, bf16 flavor: exp(SCALE*s) ~=
# bitcast_bf16(int16(s*EXPA16 + EXPB16)). The PV matmuls run in bf16 (same
# 1 cycle/row PE rate as fp32r) so the int16 bit-pattern write needs no
# f32r-rounding producer, which the BIR verifier enforces for fp32r.
EXPA16 = 0.5 * 128.0 * 1.4426950408889634
EXPB16 = 16256.0 - 5.6
# t-tile indices (mod 16) computed with the fast-exp on VectorE instead of
# ScalarE Exp. 4/16 of tiles, spread mid-chunk and kept away from t=13-15:
# the chunk-end window is when the epilogue's PSUM-release copies and
# normalize muls queue on VectorE, and a fast-exp burst there delays the
# accumulator release that gates the next chunk's PV matmuls. ScalarE has
# the slack to absorb the difference (~85us vs PE's ~105us real).
SCH = (2, 4, 7, 9, 12, 14)

_CACHE = {}


def _build_nc(reps=1, sch=SCH):
    from contextlib import ExitStack

    import concourse.bacc as bacc
    import concourse.mybir as mybir
    import concourse.tile as tile
    from concourse.masks import make_identity

    fp32 = mybir.dt.float32
    fp32r = mybir.dt.float32r
    bf16 = mybir.dt.bfloat16
    i16 = mybir.dt.int16
    Exp = mybir.ActivationFunctionType.Exp

    nc = bacc.Bacc()
    q_ext = nc.declare_dram_parameter("q", [HPC, S, D], fp32, isOutput=False)
    k_ext = nc.declare_dram_parameter("k", [HPC, S, D], fp32, isOutput=False)
    v_ext = nc.declare_dram_parameter("v", [HPC, S, D], fp32, isOutput=False)
    out_ext = nc.declare_dram_parameter("out", [HPC, S, D], fp32, isOutput=True)

    with ExitStack() as ctx:
        tc = ctx.enter_context(tile.TileContext(nc))
        consts = ctx.enter_context(tc.tile_pool(name="consts", bufs=1))
        identity = consts.tile([P, P], fp32)
        make_identity(nc, identity)
        # dummy exp: forces the ACT exp table-set DMA (~2.7us) to happen here,
        # overlapped with the input DMA lead-in, not at the first real exp.
        actwarm = consts.tile([P, 2], fp32)
        nc.scalar.activation(out=actwarm, in_=identity[:, 0:2], func=Exp, scale=1.0)


        nat = ctx.enter_context(tc.tile_pool(name="nat", bufs=2))
        vpool = ctx.enter_context(tc.tile_pool(name="vpool", bufs=2))
        qkt = ctx.enter_context(tc.tile_pool(name="qkt", bufs=2))
        ktp = ctx.enter_context(tc.tile_pool(name="ktp", bufs=2))
        ptp = ctx.enter_context(tc.tile_pool(name="ptp", bufs=10))
        otp = ctx.enter_context(tc.tile_pool(name="otp", bufs=4))
        tep = ctx.enter_context(tc.tile_pool(name="tep", bufs=2))
        op = ctx.enter_context(tc.tile_pool(name="op", bufs=2))
        rp = ctx.enter_context(tc.tile_pool(name="rp", bufs=4))
        # Epilogue O^T tiles go through the DMA xbar transpose (16-bit only),
        # staged as bf16 [OTP=80 rows, 512]: rows 0:64 = O^T, 64 = denom,
        # 65 pad, 66:80 xbar row-multiple padding (p_dim % 16 == 0).
        OTP = 80
        for _ in range(4):
            z80 = otp.tile([OTP, QCHUNK], bf16, tag="ot_sb")
            nc.gpsimd.memset(z80[D:OTP], 0.0)
        # PSUM budget (8 banks of 2KB/partition):
        #   scores [128,1024] (2 banks) x2 bufs = 4 banks, O-acc A/B = 2 banks,
        #   qk-transpose staging 1 bank, epilogue-transpose staging 1 bank.
        ps_s = ctx.enter_context(tc.tile_pool(name="ps_s", bufs=2, space="PSUM"))
        ps_oA = ctx.enter_context(tc.tile_pool(name="ps_oA", bufs=2, space="PSUM"))
        ps_oB = ctx.enter_context(tc.tile_pool(name="ps_oB", bufs=1, space="PSUM"))
        ps_t = ctx.enter_context(tc.tile_pool(name="ps_t", bufs=1, space="PSUM"))

        def prep_pair(hA, hB):
            """Emit DMA loads + V1 builds; return (state, transpose work units).

            The transpose units are emitted by the caller interleaved into the
            previous pair's ACT-bound main loop so the PE does them in slack
            cycles instead of a serial phase where ScalarE would idle.
            """
            # [p, n, r, head, d]: permuted layout; DRAM row s = n*1024+p*8+r.
            # head next-to-last so a (n,r) slice exposes a contiguous (h d)
            # 128-wide free dim for the paired transpose; DRAM-side reads
            # stay sequential within each partition's 2KB block.
            qn = nat.tile([P, NB, RB, 2, D], fp32, tag="qn", name="qn")
            kn = nat.tile([P, NB, RB, 2, D], fp32, tag="kn", name="kn")
            vn = nat.tile([P, NB, RB, 2, D], fp32, tag="vn", name="vn")
            # Load order per queue: k0, v0, q0, k1, q1, v1; heads split
            # across the two HWDGE queues so A/B blocks land in parallel.
            # v0 before q0: the z0 V1 build (GPSIMD) then finishes before the
            # first PV needs it, at the cost of ~1.2us later first-QK.
            for ext, dst, z in (
                (k_ext, kn, 0),
                (v_ext, vn, 0),
                (q_ext, qn, 0),
                (k_ext, kn, 1),
                (q_ext, qn, 1),
                (v_ext, vn, 1),
            ):
                for i, hh in enumerate((hA, hB)):
                    src = ext[hh].rearrange("(n p r) d -> p n r d", p=P, r=RB)
                    eng = nc.sync if i == 0 else nc.scalar
                    eng.dma_start(out=dst[:, z, :, i, :], in_=src[:, z])
            # V1 = [V | ones | ones] per head on GPSIMD, built per z-block
            # (z-outer, heads inner) so both heads' z0 halves are ready as
            # soon as the v-z0 DMAs land.
            v1s = [
                vpool.tile([P, T_TILES, VE], bf16, tag=f"v1{i}", name="v1")
                for i in range(2)
            ]
            for z in range(NB):
                for i in range(2):
                    zs = slice(z * RB, (z + 1) * RB)
                    nc.gpsimd.tensor_copy(
                        out=v1s[i][:, zs, 0:D], in_=vn[:, z, :, i, :]
                    )
                    nc.gpsimd.tensor_scalar(
                        out=v1s[i][:, zs, D:VE],
                        in0=vn[:, z, :, i, 0:2],
                        scalar1=0.0,
                        scalar2=1.0,
                        op0=mybir.AluOpType.mult,
                        op1=mybir.AluOpType.add,
                    )
            # Packed Q^T and K^T [128, S]: partitions 0-63 head A's [d, s],
            # 64-127 head B's, built with ONE [128,128] transpose per s-tile
            # whose free axis is (head, d). t-tile tau = n*8+r; columns
            # enumerate p. The QK matmuls run 2x ROW-TILED (64x128 mode):
            # head A on tile (0,0) reading SBUF partitions 0:64, head B on
            # tile (64,0) reading 64:128, concurrent on disjoint PE row
            # groups, writing disjoint PSUM banks (HW-verified correct incl.
            # alternation with the full-height PV matmuls).
            qt = qkt.tile([P, S], fp32r, tag="qt", name="qt")
            kt = ktp.tile([P, S], fp32r, tag="kt", name="kt")

            def unit(g, srcn, dsts):
                def emit():
                    tp = ps_t.tile([P, 4, P], fp32, tag="qk_t", name="tp")
                    for j in range(4):
                        tau = g * 4 + j
                        nc.tensor.transpose(
                            tp[:, j],
                            srcn[:, tau // RB, tau % RB].rearrange(
                                "p h d -> p (h d)"
                            ),
                            identity,
                        )
                    tpf = tp.rearrange("p a b -> p (a b)")
                    for dst, p0, p1 in dsts:
                        nc.vector.tensor_copy(
                            out=dst[p0:p1, g * 512 : (g + 1) * 512],
                            in_=tpf[p0:p1],
                        )
                return emit

            ku = [unit(g, kn, ((kt, 0, P),)) for g in range(4)]
            qu = [unit(g, qn, ((qt, 0, P),)) for g in range(4)]
            # Order so a 3-unit upfront prefix = [Kg0, Kg1, Qg0] — exactly
            # what the first q-chunk's first t-tiles need, fed by the first
            # two DMAs per queue.
            units = [ku[0], ku[1], qu[0], ku[2], ku[3], qu[1], qu[2], qu[3]]
            return (qt, kt, v1s), units

        pair_seq = [
            (2 * pr, 2 * pr + 1) for _ in range(reps) for pr in range(HPC // 2)
        ]
        state, units = prep_pair(*pair_seq[0])
        # Pair 0 has no previous loop to hide its transposes in: emit just
        # [Kg0, Kg1, Qg0] upfront (ready after ~2 DMAs per queue); the rest
        # drop into its own loop slots from git>=4, by which time the z1
        # loads have landed.
        n_upfront = 3
        for u in units[:n_upfront]:
            u()
        units = units[n_upfront:]
        for pi, (hA, hB) in enumerate(pair_seq):
            for u in units[: max(0, len(units) - 16)]:
                u()  # leftovers beyond one pair's absorption capacity
            units = units[max(0, len(units) - 16) :]
            qt, kt, v1s = state
            # whole-pair output staging: one 512KB store per head at pair
            # end (2KB DRAM chunks) instead of eight 128KB per-qc stores.
            o_heads = [
                op.tile([P, NB, RB, D], fp32, tag=f"o_h{i}", name=f"o_h{i}")
                for i in range(2)
            ]
            next_units = []
            if pi + 1 < len(pair_seq):
                state, next_units = prep_pair(*pair_seq[pi + 1])
            units = units + next_units

            # One global iteration stream per pair: PV lags the scores by
            # 2 iterations ACROSS q-chunk boundaries, so the PV pipeline
            # never drains/refills at the 3 interior boundaries. The
            # epilogue is split: the PSUM-release copies issue right after
            # pv(qc,15); the PE transposes+normalize run 4 iterations later
            # (slot t=5 of the next chunk) so they never block the next
            # scores matmuls on the copy in PE program order.
            o_by_qc = {}
            pts = {}
            ot_by_qc = {}
            NTOT = N_QCHUNKS * T_TILES

            def pv(git):
                pqc, pt_ = divmod(git, T_TILES)
                o_pss = o_by_qc[pqc]
                ptile = pts.pop(git)
                for i in range(2):
                    nc.tensor.matmul(
                        o_pss[i],
                        lhsT=v1s[i][:, pt_],
                        rhs=ptile[:, i * QCHUNK : (i + 1) * QCHUNK],
                        start=(pt_ == 0),
                        stop=(pt_ == T_TILES - 1),
                    )

            def ep_copies(qc, last=False):
                o_pss = o_by_qc[qc]
                obs = []
                for i in range(2):
                    ot_sb = otp.tile([OTP, QCHUNK], bf16, tag="ot_sb")
                    nc.vector.tensor_copy(out=ot_sb[0:D], in_=o_pss[i][0:D])
                    # reciprocal of the denominator computed BEFORE the
                    # transpose (from full-precision PSUM), carried through
                    # the xbar in rows 64:66 so ep_late needs no DVE work
                    # that would queue behind the DMA chain. (V1's ones-build
                    # fills cols 64 AND 65, so PSUM rows 64/65 both hold the
                    # denominator; 2-partition block keeps the base aligned.)
                    with nc.allow_low_precision(reason="bf16 recip row"):
                        nc.vector.reciprocal(
                            out=ot_sb[D : D + 2, :], in_=o_pss[i][D : D + 2, :]
                        )
                    # single 3D-out xbar transpose [80, 512] -> [128, 4, 80]
                    # on the SP queue: a DMA instruction that waits (here: on
                    # the DVE copy) blocks its issuing engine's sequencer,
                    # and on the ACT queue that would stall the exp stream.
                    te = tep.tile([P, 4, OTP], bf16, tag="te_sb")
                    # final qc of the run: nothing left on the ACT stream to
                    # block, so split the two heads across both queues to
                    # halve the tail's transpose latency
                    teng = nc.scalar if (last and i == 1) else nc.sync
                    teng.dma_start(out=te, in_=ot_sb, transpose=True)
                    obs.append((ot_sb, te))
                ot_by_qc[qc] = obs
                del o_by_qc[qc]

            def ep_late(qc, last=False):
                nn = (4 * qc) // RB
                r0 = (4 * qc) % RB
                for i in range(2):
                    _, te = ot_by_qc[qc][i]
                    # normalize on GPSIMD (per-partition recip in te col 65):
                    # keeps the te dependency off DVE/ACT so a slow DMA chain
                    # can never stall the exp pipeline. (gpsimd tensor_scalar
                    # needs an fp32 scalar AP -> tiny widening copy first)
                    # Final qc: head A on the now-idle DVE to halve the tail.
                    eng = nc.vector if (last and i == 0) else nc.gpsimd
                    rec32 = rp.tile([P, 4], fp32, tag="rec32")
                    eng.tensor_copy(out=rec32, in_=te[:, :, D + 1])
                    for j in range(4):
                        eng.tensor_scalar_mul(
                            o_heads[i][:, nn, r0 + j],
                            te[:, j, 0:D],
                            rec32[:, j : j + 1],
                        )
                del ot_by_qc[qc]

            def scores(git):
                qc, t = divmod(git, T_TILES)
                if t == 0:
                    o_by_qc[qc] = (
                        ps_oA.tile([VE, QCHUNK], fp32, tag="o_accA", name="o_psA"),
                        ps_oB.tile([VE, QCHUNK], fp32, tag="o_accB", name="o_psB"),
                    )
                qsl = slice(qc * QCHUNK, (qc + 1) * QCHUNK)
                tsl = slice(t * P, (t + 1) * P)
                s_ps = ps_s.tile([P, 2 * QCHUNK], fp32, tag="scores", name="s_ps")
                # 2x row-tiled (64x128 mode): heads on disjoint PE row groups
                nc.tensor.matmul(
                    s_ps[:, 0:QCHUNK], lhsT=kt[0:D, tsl], rhs=qt[0:D, qsl],
                    start=True, stop=True, tile_position=(0, 0),
                )
                nc.tensor.matmul(
                    s_ps[:, QCHUNK:], lhsT=kt[D:P, tsl], rhs=qt[D:P, qsl],
                    start=True, stop=True, tile_position=(64, 0),
                )
                pt = ptp.tile([P, 2 * QCHUNK], bf16, tag="pt", name="pt")
                pts[git] = pt
                if t % 16 in sch:
                    # VectorE fast-exp (Schraudolph, bf16 flavor):
                    # s*EXPA16+EXPB16 -> int16 bits of bf16(~exp(0.5 s))
                    nc.vector.tensor_scalar(
                        out=pt.bitcast(i16),
                        in0=s_ps,
                        scalar1=EXPA16,
                        scalar2=EXPB16,
                        op0=mybir.AluOpType.mult,
                        op1=mybir.AluOpType.add,
                    )
                else:
                    nc.scalar.activation(out=pt, in_=s_ps, func=Exp, scale=SCALE)

            # Mode-switch grouping: the tiled scores matmuls and the
            # full-height PV/transpose matmuls use different PE tiling
            # modes, and each mode switch costs a drain (~0.2us measured).
            # Two score tiles per switch (the PSUM scores-buffer limit)
            # amortizes it: [QK(g) QK(g+1)] tiled | [PV(g-2) PV(g-1) + one
            # transpose unit] full-height.
            for git in range(NTOT + 2 + 4):
                if git < NTOT and git % 2 == 0:
                    scores(git)
                    scores(git + 1)
                lag = git - 2
                if 0 <= lag < NTOT and lag % 2 == 0:
                    pv(lag)
                    pv(lag + 1)
                    if (lag + 1) % T_TILES == T_TILES - 1:
                        lqc = (lag + 1) // T_TILES
                        ep_copies(
                            lqc,
                            last=(
                                pi == len(pair_seq) - 1
                                and lqc == N_QCHUNKS - 1
                            ),
                        )
                if git < NTOT and git % 2 == 0:
                    t = git % T_TILES
                    # absorb pending transpose units in the full-height
                    # cluster (pair 0: not before git 4, so its z1-fed units
                    # don't stall PE while those DMAs are still inbound)
                    if t % 4 in (0, 2) and units and (pi > 0 or git >= 4):
                        units.pop(0)()
                if git >= 21 and (git - 21) % T_TILES == 0:
                    qe = (git - 21) // T_TILES
                    if qe < N_QCHUNKS:
                        ep_late(
                            qe,
                            last=(
                                pi == len(pair_seq) - 1
                                and qe == N_QCHUNKS - 1
                            ),
                        )
                        if qe % 2 == 1:
                            # store the finished half (n-block) of each head:
                            # [128, 8, 64] = 2KB/partition DRAM chunks, heads
                            # split across the two HWDGE queues.
                            nnq = qe // 2
                            for i, hh in enumerate((hA, hB)):
                                eng = nc.sync
                                eng.dma_start(
                                    out=out_ext[hh].rearrange(
                                        "(n p r) d -> p n r d", p=P, r=RB
                                    )[:, nnq],
                                    in_=o_heads[i][:, nnq],
                                )
    nc.finalize()
    return nc


def _get_nc(reps=1, sch=SCH):
    key = f"nc{reps}s{sch}"
    if key not in _CACHE:
        _CACHE[key] = _build_nc(reps, sch=sch)
    return _CACHE[key]


def _shard(x):
    x = np.ascontiguousarray(np.asarray(x), dtype=np.float32).reshape(B * H, S, D)
    return [np.ascontiguousarray(x[i * HPC : (i + 1) * HPC]) for i in range(N_CORES)]


def run(q, k, v, trace=False, **kw):
    from concourse.bass_utils import run_bass_kernel_spmd

    qs, ks, vs = _shard(q), _shard(k), _shard(v)
    in_maps = [{"q": qs[i], "k": ks[i], "v": vs[i]} for i in range(N_CORES)]
    res = run_bass_kernel_spmd(
        _get_nc(), in_maps, core_ids=list(range(N_CORES)), trace=trace, **kw
    )
    out = np.concatenate([res.results[i]["out"] for i in range(N_CORES)], axis=0)
    return out.reshape(B, H, S, D), res


def kernel(q, k, v):
    out, _ = run(q, k, v)
    return out

